# revision 1
# baseline (speedup 1.0000x reference)
"""GCN layer (GCNConv on a fully-connected 4096-node graph) on 8 trn2 NeuronCores.

Math (see harness reference):
    A[i, j] = edge_weights[i*4096 + j]          (edge_index is the full meshgrid)
    deg[j]  = sum_i A[i, j]
    d       = deg ** -0.5                        (deg > 0 always here)
    h       = x @ W
    out     = d[:, None] * (A.T @ (d[:, None] * h)) + b

Sharding: tensor-parallel over the feature dim. Core c owns 256 of the 2048
output features: computes h[:, fs], scales by d, then
outT[f, j] = sum_i z[i, f] * A[i, j] via PE matmuls (z tiles stationary,
A streamed), finally scales by d[j] and adds b. Host concatenates shards.

Phases on PE: DEG (fp8 copy of A streamed through ones-stationary matmuls)
-> H (x @ W, bf16) -> AGG (bf16 A streamed once). The d = rsqrt(deg) chain
(ACT sqrt, DVE reciprocal, DRAM-roundtrip relayout) overlaps the H phase.
All matmul accumulation is fp32 in PSUM.
"""

import sys

sys.path.insert(0, "/opt/trn_rl_repo")

import numpy as np
import ml_dtypes

N = 4096          # nodes
K = 2048          # num_kernels (features)
F = 256           # features per core (2048 / 8)
NB = N // 128     # 32 node blocks
KB = K // 128     # 16 contraction blocks
NG = N // 512     # 8 column groups of 512
P = 128

_BF16 = ml_dtypes.bfloat16
_FP8 = ml_dtypes.float8_e4m3fn
_cache = {}


def _build():
    import concourse.bass as bass
    import concourse.mybir as mybir
    from concourse import bacc
    from concourse.tile import TileContext

    dt = mybir.dt
    nc = bacc.Bacc("TRN2", target_bir_lowering=False)

    A = nc.dram_tensor("A", [N, N], dt.bfloat16, kind="ExternalInput")
    A8 = nc.dram_tensor("A8", [N, N], dt.float8e4, kind="ExternalInput")
    xTb = nc.dram_tensor("xTb", [8, P, KB, 512], dt.bfloat16, kind="ExternalInput")
    Wt = nc.dram_tensor("Wt", [K, F], dt.bfloat16, kind="ExternalInput")
    bs = nc.dram_tensor("bs", [F], dt.float32, kind="ExternalInput")
    outT = nc.dram_tensor("outT", [F, N], dt.float32, kind="ExternalOutput")
    dscr = nc.dram_tensor("dscr", [N], dt.float32)

    with TileContext(nc) as tc:
        with (
            tc.tile_pool(name="const", bufs=1) as const,
            tc.tile_pool(name="xt", bufs=4) as xt_pool,
            tc.tile_pool(name="w", bufs=1) as w_pool,
            tc.tile_pool(name="hz", bufs=1) as hz_pool,
            tc.tile_pool(name="a8", bufs=4) as a8_pool,
            tc.tile_pool(name="a2", bufs=14) as a2_pool,
            tc.tile_pool(name="ev", bufs=4) as ev_pool,
            tc.tile_pool(name="ps", bufs=8, space="PSUM") as ps,
        ):
            ones8 = const.tile([P, P], dt.float8e4)
            nc.vector.memset(ones8, 1.0)

            # ---- Phase DEG: deg[j] = sum_i A[i, j] from the fp8 copy of A.
            degp = [
                ps.tile([P, 512], dt.float32, tag="ps", name=f"degp{g}")
                for g in range(NG)
            ]
            for ibb in range(NB // 2):  # pairs of 128-row blocks
                a8_t = a8_pool.tile([P, 2, N], dt.float8e4)
                nc.sync.dma_start(
                    out=a8_t,
                    in_=bass.AP(
                        tensor=A8,
                        offset=ibb * 2 * P * N,
                        ap=[[N, P], [P * N, 2], [1, N]],
                    ),
                )
                for blk in range(2):
                    for g in range(NG):
                        nc.tensor.matmul(
                            degp[g],
                            ones8,
                            a8_t[:, blk, g * 512:(g + 1) * 512],
                            start=(ibb == 0 and blk == 0),
                            stop=(ibb == NB // 2 - 1 and blk == 1),
                        )

            # d = 1/sqrt(deg): ACT Sqrt (reads PSUM row 0), DVE reciprocal,
            # then a DRAM round-trip to relayout into partition + bcast forms.
            # All of this overlaps the H phase on PE.
            d_row = const.tile([1, N], dt.float32)
            for g in range(NG):
                nc.scalar.activation(
                    out=d_row[0:1, g * 512:(g + 1) * 512],
                    in_=degp[g][0:1, :],
                    func=mybir.ActivationFunctionType.Sqrt,
                )
            # ---- Phase H: h[:, fs] = x @ W[:, fs]; z = d[i] * h (bf16)
            w_sb = w_pool.tile([P, KB, F], dt.bfloat16)
            nc.sync.dma_start(
                out=w_sb,
                in_=bass.AP(tensor=Wt, offset=0, ap=[[F, P], [P * F, KB], [1, F]]),
            )
            b_col = const.tile([P, 2], dt.float32)
            for fh in range(2):
                nc.gpsimd.dma_start(
                    out=b_col[:, fh:fh + 1],
                    in_=bs[fh * P:(fh + 1) * P].rearrange("(p o) -> p o", o=1),
                )

            z_sb = hz_pool.tile([P, NB, F], dt.bfloat16)
            for ic in range(8):  # i-chunks of 512 nodes
                xt_t = xt_pool.tile([P, KB, 512], dt.bfloat16)
                nc.sync.dma_start(out=xt_t, in_=xTb[ic])
                for ii in range(4):
                    ib = ic * 4 + ii
                    hp = ps.tile([P, 512], dt.float32, tag="ps")
                    for kb in range(KB):
                        nc.tensor.matmul(
                            hp[:, :F],
                            xt_t[:, kb, ii * P:(ii + 1) * P],
                            w_sb[:, kb, :],
                            start=(kb == 0),
                            stop=(kb == KB - 1),
                        )
                    nc.vector.tensor_copy(z_sb[:, ib, :], hp[:, :F])

            # d_row holds sqrt(deg). Relayout FIRST (DRAM round-trip), then
            # take reciprocals in partition-parallel form — a [1, 4096]
            # single-partition DVE reciprocal measures ~25us and would block
            # the in-order DVE stream mid-H.
            nc.scalar.dma_start(out=dscr[:].rearrange("(o j) -> o j", o=1), in_=d_row)
            st = const.tile([P, NB], dt.float32)
            nc.scalar.dma_start(
                out=st, in_=bass.AP(tensor=dscr, offset=0, ap=[[1, P], [P, NB]])
            )
            sbc = const.tile([P, N], dt.float32)
            nc.scalar.dma_start(
                out=sbc, in_=bass.AP(tensor=dscr, offset=0, ap=[[0, P], [1, N]])
            )
            d_t = const.tile([P, NB], dt.float32)
            nc.vector.reciprocal(d_t, st)
            d_bc = sbc  # reciprocal in place, 512-col chunks
            for g in range(NG):
                nc.vector.reciprocal(
                    d_bc[:, g * 512:(g + 1) * 512], sbc[:, g * 512:(g + 1) * 512]
                )

            # z = d[i] * h in place (waits on d_t, but holds no PSUM)
            for ib in range(NB):
                nc.vector.tensor_scalar_mul(
                    z_sb[:, ib, :], z_sb[:, ib, :], d_t[:, ib:ib + 1]
                )

            # ---- Phase AGG: outT[f, j] = sum_i z[i, f] A[i, j], *d[j] + b.
            # Four j-quarter passes; each holds 4 PSUM banks (2 jg x 2 fh) so
            # consecutive passes double-buffer through the 8-bank pool.
            for q in range(4):
                op = [
                    ps.tile([P, 512], dt.float32, tag="ps", name=f"op{q}_{t}")
                    for t in range(4)
                ]  # index: jh * 2 + fh, jh in {0,1} within the quarter
                for ib in range(NB):
                    a2 = a2_pool.tile([P, 1024], dt.bfloat16)
                    nc.sync.dma_start(
                        out=a2,
                        in_=A[ib * P:(ib + 1) * P, q * 1024:(q + 1) * 1024],
                    )
                    for fh in range(2):
                        for jh in range(2):
                            nc.tensor.matmul(
                                op[jh * 2 + fh],
                                z_sb[:, ib, fh * P:(fh + 1) * P],
                                a2[:, jh * 512:(jh + 1) * 512],
                                start=(ib == 0),
                                stop=(ib == NB - 1),
                            )
                for jh in range(2):
                    for fh in range(2):
                        jg = q * 2 + jh
                        ev = ev_pool.tile([P, 512], dt.float32)
                        nc.vector.tensor_mul(
                            ev, op[jh * 2 + fh], d_bc[:, jg * 512:(jg + 1) * 512]
                        )
                        nc.vector.tensor_scalar_add(ev, ev, b_col[:, fh:fh + 1])
                        nc.scalar.dma_start(
                            out=outT[fh * P:(fh + 1) * P, jg * 512:(jg + 1) * 512],
                            in_=ev,
                        )

    nc.compile()
    return nc


def _get_nc():
    if "nc" not in _cache:
        _cache["nc"] = _build()
    return _cache["nc"]


def _prep_inputs(x, edge_weights, W, b):
    A32 = np.asarray(edge_weights, np.float32).reshape(N, N)
    A16 = A32.astype(_BF16)
    A8 = A32.astype(_FP8)
    x32 = np.asarray(x, np.float32)
    # xTb[ic, p, kb, i] = x[ic*512 + i, kb*128 + p]
    xTb = np.ascontiguousarray(
        x32.reshape(8, 512, KB, P).transpose(0, 3, 2, 1).astype(_BF16)
    )
    W16 = np.asarray(W, np.float32).astype(_BF16)
    b32 = np.ascontiguousarray(np.asarray(b, np.float32))
    in_maps = []
    for c in range(8):
        in_maps.append(
            {
                "A": A16,
                "A8": A8,
                "xTb": xTb,
                "Wt": np.ascontiguousarray(W16[:, c * F:(c + 1) * F]),
                "bs": np.ascontiguousarray(b32[c * F:(c + 1) * F]),
            }
        )
    return in_maps


def _run(in_maps, trace=False):
    from concourse.bass_utils import run_bass_kernel_spmd

    nc = _get_nc()
    return run_bass_kernel_spmd(nc, in_maps, list(range(8)), trace=trace)


def kernel(x, edge_index, edge_weights, W, b):
    in_maps = _prep_inputs(x, edge_weights, W, b)
    res = _run(in_maps)
    out = np.empty((N, K), np.float32)
    for c in range(8):
        out[:, c * F:(c + 1) * F] = np.asarray(res.results[c]["outT"]).T
    return out



# revision 2
# speedup vs baseline: 1.3715x; 1.3715x over previous
"""GCN layer (GCNConv on a fully-connected 4096-node graph) on 8 trn2 NeuronCores.

Math (see harness reference):
    A[i, j] = edge_weights[i*4096 + j]          (edge_index is the full meshgrid)
    deg[j]  = sum_i A[i, j]
    d       = deg ** -0.5                        (deg > 0 always here)
    An      = d[:, None] * A * d[None, :]        (symmetric normalization)
    out     = An.T @ (x @ W) + b

Sharding: tensor-parallel over the feature dim; An is replicated (as the
sharding hint suggests). The normalization (deg, rsqrt, row/col scaling) is
folded into An on the host during the bf16 cast, so the device kernel is two
back-to-back GEMM phases per core c (owning 256 of the 2048 features):
    H:   h = x @ W[:, fs]            (x^T blocks stationary, W moving)
    AGG: outT[f, j] = sum_i h[i, f] * An[i, j]   (h blocks stationary, An moving)
All accumulation fp32 in PSUM; An streamed once as bf16 in contiguous 2MB
chunks. b is added on the host (it is all-zeros in this model anyway).
"""

import sys

sys.path.insert(0, "/opt/trn_rl_repo")

import numpy as np
import ml_dtypes

N = 4096          # nodes
K = 2048          # num_kernels (features)
F = 256           # features per core (2048 / 8)
NB = N // 128     # 32 node blocks
KB = K // 128     # 16 contraction blocks
P = 128

_BF16 = ml_dtypes.bfloat16
_cache = {}


def _build():
    import concourse.bass as bass
    import concourse.mybir as mybir
    from concourse import bacc
    from concourse.tile import TileContext

    dt = mybir.dt
    nc = bacc.Bacc("TRN2", target_bir_lowering=False)

    # Apk[q, cc, p, t, j] = An[(cc*8+t)*128 + p, q*1024 + j]; each [q, cc] slice
    # is a contiguous 2MB chunk covering 8 i-blocks x 1024 j-columns.
    Apk = nc.dram_tensor("Apk", [4, 4, P, 8, 1024], dt.bfloat16, kind="ExternalInput")
    # xTb[ic, p, kb, i] = x[ic*512 + i, kb*128 + p]
    xTb = nc.dram_tensor("xTb", [8, P, KB, 512], dt.bfloat16, kind="ExternalInput")
    Wt = nc.dram_tensor("Wt", [K, F], dt.bfloat16, kind="ExternalInput")
    # outTb[fh, jg, p, j] = outT[fh*128 + p, jg*512 + j]
    outTb = nc.dram_tensor("outTb", [2, 8, P, 512], dt.float32, kind="ExternalOutput")

    with TileContext(nc) as tc:
        with (
            tc.tile_pool(name="w", bufs=1) as w_pool,
            tc.tile_pool(name="xt", bufs=3) as xt_pool,
            tc.tile_pool(name="z", bufs=1) as z_pool,
            tc.tile_pool(name="a", bufs=3) as a_pool,
            tc.tile_pool(name="ev", bufs=4) as ev_pool,
            tc.tile_pool(name="ps", bufs=8, space="PSUM") as ps,
        ):
            # W in 4 kb-group chunks (scalar queue, ahead of the An stream there)
            w_sb = w_pool.tile([P, KB, F], dt.bfloat16)
            for g in range(4):
                nc.scalar.dma_start(
                    out=w_sb[:, 4 * g:4 * g + 4, :],
                    in_=bass.AP(
                        tensor=Wt,
                        offset=4 * g * P * F,
                        ap=[[F, P], [P * F, 4], [1, F]],
                    ),
                )

            z_sb = z_pool.tile([P, NB, F], dt.bfloat16)

            # ---- Phase H: h[:, fs] = x @ W[:, fs]
            # ic = 0 runs kb-group-outer so the PE starts after the first 512KB
            # chunk instead of waiting for the full 2MB tile.
            xt0 = xt_pool.tile([P, KB, 512], dt.bfloat16)
            for g in range(4):
                nc.sync.dma_start(
                    out=xt0[:, 4 * g:4 * g + 4, :],
                    in_=bass.AP(
                        tensor=xTb,
                        offset=4 * g * 512,
                        ap=[[KB * 512, P], [512, 4], [1, 512]],
                    ),
                )
            hp0 = [ps.tile([P, 512], dt.float32, tag="ps", name=f"hp0_{ii}")
                   for ii in range(4)]
            for g in range(4):
                for ii in range(4):
                    for kb in range(4 * g, 4 * g + 4):
                        nc.tensor.matmul(
                            hp0[ii][:, :F],
                            xt0[:, kb, ii * P:(ii + 1) * P],
                            w_sb[:, kb, :],
                            start=(kb == 0),
                            stop=(kb == KB - 1),
                        )
            for ii in range(4):
                nc.vector.tensor_copy(z_sb[:, ii, :], hp0[ii][:, :F])

            for ic in range(1, 8):
                xt = xt_pool.tile([P, KB, 512], dt.bfloat16)
                nc.sync.dma_start(out=xt, in_=xTb[ic])
                for ii in range(4):
                    ib = ic * 4 + ii
                    hp = ps.tile([P, 512], dt.float32, tag="ps")
                    for kb in range(KB):
                        nc.tensor.matmul(
                            hp[:, :F],
                            xt[:, kb, ii * P:(ii + 1) * P],
                            w_sb[:, kb, :],
                            start=(kb == 0),
                            stop=(kb == KB - 1),
                        )
                    nc.vector.tensor_copy(z_sb[:, ib, :], hp[:, :F])

            # ---- Phase AGG: outT[f, j] = sum_i h[i, f] An[i, j]
            # Four j-quarter passes, 4 PSUM banks each; An streamed once in
            # 2MB contiguous chunks (4 per quarter).
            for q in range(4):
                op = [ps.tile([P, 512], dt.float32, tag="ps", name=f"op{q}_{t}")
                      for t in range(4)]  # index jh*2 + fh
                for cc in range(4):
                    a_t = a_pool.tile([P, 8, 1024], dt.bfloat16)
                    nc.scalar.dma_start(out=a_t, in_=Apk[q, cc])
                    for t in range(8):
                        ib = cc * 8 + t
                        for fh in range(2):
                            for jh in range(2):
                                nc.tensor.matmul(
                                    op[jh * 2 + fh],
                                    z_sb[:, ib, fh * P:(fh + 1) * P],
                                    a_t[:, t, jh * 512:(jh + 1) * 512],
                                    start=(ib == 0),
                                    stop=(ib == NB - 1),
                                )
                for jh in range(2):
                    for fh in range(2):
                        jg = q * 2 + jh
                        ev = ev_pool.tile([P, 512], dt.float32)
                        nc.vector.tensor_copy(ev, op[jh * 2 + fh])
                        nc.sync.dma_start(
                            out=bass.AP(
                                tensor=outTb,
                                offset=(fh * 8 + jg) * P * 512,
                                ap=[[512, P], [1, 512]],
                            ),
                            in_=ev,
                        )

    nc.compile()
    return nc


def _get_nc():
    if "nc" not in _cache:
        _cache["nc"] = _build()
    return _cache["nc"]


def _prep_inputs(x, edge_weights, W, b):
    A32 = np.asarray(edge_weights, np.float32).reshape(N, N)
    deg = A32.sum(axis=0, dtype=np.float64)
    d = 1.0 / np.sqrt(deg)
    An = (A32 * d[None, :].astype(np.float32)) * d[:, None].astype(np.float32)
    An16 = An.astype(_BF16)
    # Apk[q, cc, p, t, j] = An[(cc*8+t)*128+p, q*1024+j]
    Apk = np.ascontiguousarray(
        An16.reshape(4, 8, P, 4, 1024).transpose(3, 0, 2, 1, 4)
    )
    x32 = np.asarray(x, np.float32)
    # xTb[ic, p, kb, i] = x[ic*512 + i, kb*128 + p]
    xTb = np.ascontiguousarray(
        x32.reshape(8, 512, KB, P).transpose(0, 3, 2, 1).astype(_BF16)
    )
    W16 = np.asarray(W, np.float32).astype(_BF16)
    in_maps = []
    for c in range(8):
        in_maps.append(
            {
                "Apk": Apk,
                "xTb": xTb,
                "Wt": np.ascontiguousarray(W16[:, c * F:(c + 1) * F]),
            }
        )
    return in_maps


def _run(in_maps, trace=False):
    from concourse.bass_utils import run_bass_kernel_spmd

    nc = _get_nc()
    return run_bass_kernel_spmd(nc, in_maps, list(range(8)), trace=trace)


def kernel(x, edge_index, edge_weights, W, b):
    in_maps = _prep_inputs(x, edge_weights, W, b)
    res = _run(in_maps)
    out = np.empty((N, K), np.float32)
    for c in range(8):
        # outTb [2, 8, 128, 512] -> outT [256, 4096]
        outT = (
            np.asarray(res.results[c]["outTb"])
            .transpose(0, 2, 1, 3)
            .reshape(F, N)
        )
        out[:, c * F:(c + 1) * F] = outT.T
    out += np.asarray(b, np.float32)[None, :]
    return out


# revision 3
# speedup vs baseline: 1.4062x; 1.0253x over previous
"""GCN layer (GCNConv on a fully-connected 4096-node graph) on 8 trn2 NeuronCores.

Math (see harness reference):
    A[i, j] = edge_weights[i*4096 + j]          (edge_index is the full meshgrid)
    deg[j]  = sum_i A[i, j]
    d       = deg ** -0.5                        (deg > 0 always here)
    An      = d[:, None] * A * d[None, :]        (symmetric normalization)
    out     = An.T @ (x @ W) + b

Sharding: row-parallel (the sharding hint's alternative): core c owns rows
i in [c*512, (c+1)*512) of An and x. The normalization is folded into An on
the host during the bf16 cast. Each core computes
    h_c = x_c @ W                  (512 x 2048, full W streamed)
    P_c[f, j] = sum_{i in shard} h_c[i, f] * An[i, j]
and the host sums the 8 partials (the "all-reduce" of the hint) and adds b.
An_c (4MB) is fully SBUF-resident, so the AGG phase reads nothing from HBM
except writing the bf16 partial out; no DMA stream exceeds ~50% of HBM BW.
All accumulation fp32 in PSUM.
"""

import sys

sys.path.insert(0, "/opt/trn_rl_repo")

import numpy as np
import ml_dtypes

N = 4096          # nodes
K = 2048          # num_kernels (features)
R = 512           # rows per core (4096 / 8)
RB = R // 128     # 4 row blocks per core
KB = K // 128     # 16 contraction blocks
FG = K // 512     # 4 f-groups of 512
JG = N // 512     # 8 j-groups of 512
P = 128

_BF16 = ml_dtypes.bfloat16
_cache = {}


def _build():
    import concourse.bass as bass
    import concourse.mybir as mybir
    from concourse import bacc
    from concourse.tile import TileContext

    dt = mybir.dt
    nc = bacc.Bacc("TRN2", target_bir_lowering=False)

    # Ans[p, ib, j] = An[c*512 + ib*128 + p, j] (this core's row shard)
    Ans = nc.dram_tensor("Ans", [P, RB, N], dt.bfloat16, kind="ExternalInput")
    # xTs[p, kb, i] = x[c*512 + i, kb*128 + p]
    xTs = nc.dram_tensor("xTs", [P, KB, R], dt.bfloat16, kind="ExternalInput")
    # Wb[fg, p, kb, f'] = W[kb*128 + p, fg*512 + f']
    Wb = nc.dram_tensor("Wb", [FG, P, KB, 512], dt.bfloat16, kind="ExternalInput")
    # outPb[jg, sg, p, s, j] = P_c[(sg*4+s)*128 + p, jg*512 + j]
    outPb = nc.dram_tensor("outPb", [JG, 4, P, 4, 512], dt.bfloat16,
                           kind="ExternalOutput")

    with TileContext(nc) as tc:
        with (
            tc.tile_pool(name="xt", bufs=1) as xt_pool,
            tc.tile_pool(name="an", bufs=1) as an_pool,
            tc.tile_pool(name="w", bufs=2) as w_pool,
            tc.tile_pool(name="z", bufs=1) as z_pool,
            tc.tile_pool(name="st", bufs=3) as st_pool,
            tc.tile_pool(name="ps", bufs=8, space="PSUM") as ps,
        ):
            # x^T shard: 4 x 512KB chunks on sync so the PE can start early
            xt = xt_pool.tile([P, KB, R], dt.bfloat16)
            for g in range(4):
                nc.sync.dma_start(
                    out=xt[:, 4 * g:4 * g + 4, :],
                    in_=bass.AP(
                        tensor=xTs,
                        offset=4 * g * R,
                        ap=[[KB * R, P], [R, 4], [1, R]],
                    ),
                )
            # An shard: 4MB in 2 chunks on sync, lands well before AGG starts
            an = an_pool.tile([P, RB, N], dt.bfloat16)
            for g in range(2):
                nc.sync.dma_start(
                    out=an[:, 2 * g:2 * g + 2, :],
                    in_=bass.AP(
                        tensor=Ans,
                        offset=2 * g * N,
                        ap=[[RB * N, P], [N, 2], [1, N]],
                    ),
                )

            z_sb = z_pool.tile([P, RB, K], dt.bfloat16)

            # ---- Phase H: h = x_c @ W, f-group slabs so W streams evenly.
            for fg in range(FG):
                w_t = w_pool.tile([P, KB, 512], dt.bfloat16)
                if fg == 0:  # first slab in 4 chunks so the PE starts earlier
                    for g in range(4):
                        nc.scalar.dma_start(
                            out=w_t[:, 4 * g:4 * g + 4, :],
                            in_=bass.AP(
                                tensor=Wb,
                                offset=4 * g * 512,
                                ap=[[KB * 512, P], [512, 4], [1, 512]],
                            ),
                        )
                else:
                    nc.scalar.dma_start(out=w_t, in_=Wb[fg])
                hp = [ps.tile([P, 512], dt.float32, tag="ps", name=f"hp{fg}_{ib}")
                      for ib in range(RB)]
                for kb in range(KB):
                    for ib in range(RB):
                        nc.tensor.matmul(
                            hp[ib],
                            xt[:, kb, ib * P:(ib + 1) * P],
                            w_t[:, kb, :],
                            start=(kb == 0),
                            stop=(kb == KB - 1),
                        )
                for ib in range(RB):
                    nc.vector.tensor_copy(z_sb[:, ib, fg * 512:(fg + 1) * 512],
                                          hp[ib])

            # ---- Phase AGG: P_c[f, j] = sum_i z[i, f] An[i, j]
            # 128 psum groups of 4 accumulating matmuls; evictions staged in
            # SBUF (bf16) and written out 512KB at a time.
            for jg in range(JG):
                for sg in range(4):
                    stage = st_pool.tile([P, 4, 512], dt.bfloat16)
                    for s in range(4):
                        fh = sg * 4 + s
                        op = ps.tile([P, 512], dt.float32, tag="ps")
                        for ib in range(RB):
                            nc.tensor.matmul(
                                op,
                                z_sb[:, ib, fh * P:(fh + 1) * P],
                                an[:, ib, jg * 512:(jg + 1) * 512],
                                start=(ib == 0),
                                stop=(ib == RB - 1),
                            )
                        nc.vector.tensor_copy(stage[:, s, :], op)
                    nc.sync.dma_start(
                        out=outPb[jg, sg],
                        in_=stage,
                    )

    nc.compile()
    return nc


def _get_nc():
    if "nc" not in _cache:
        _cache["nc"] = _build()
    return _cache["nc"]


def _prep_inputs(x, edge_weights, W, b):
    A32 = np.asarray(edge_weights, np.float32).reshape(N, N)
    deg = A32.sum(axis=0, dtype=np.float64)
    d = 1.0 / np.sqrt(deg)
    An = (A32 * d[None, :].astype(np.float32)) * d[:, None].astype(np.float32)
    An16 = An.astype(_BF16)
    x16 = np.asarray(x, np.float32).astype(_BF16)
    W16 = np.asarray(W, np.float32).astype(_BF16)
    # Wb[fg, p, kb, f'] = W[kb*128+p, fg*512+f']  (shared by all cores)
    Wb = np.ascontiguousarray(
        W16.reshape(KB, P, FG, 512).transpose(2, 1, 0, 3)
    )
    in_maps = []
    for c in range(8):
        rows = slice(c * R, (c + 1) * R)
        # Ans[p, ib, j] = An[c*512 + ib*128 + p, j]
        Ans = np.ascontiguousarray(
            An16[rows].reshape(RB, P, N).transpose(1, 0, 2)
        )
        # xTs[p, kb, i] = x[c*512 + i, kb*128 + p]
        xTs = np.ascontiguousarray(
            x16[rows].reshape(R, KB, P).transpose(2, 1, 0)
        )
        in_maps.append({"Ans": Ans, "xTs": xTs, "Wb": Wb})
    return in_maps


def _run(in_maps, trace=False):
    from concourse.bass_utils import run_bass_kernel_spmd

    nc = _get_nc()
    return run_bass_kernel_spmd(nc, in_maps, list(range(8)), trace=trace)


def kernel(x, edge_index, edge_weights, W, b):
    in_maps = _prep_inputs(x, edge_weights, W, b)
    res = _run(in_maps)
    # host-side all-reduce of the 8 row-shard partials
    acc = np.zeros((K, N), np.float32)
    for c in range(8):
        # outPb [8, 4, 128, 4, 512] -> P_c [2048, 4096]
        Pc = (
            np.asarray(res.results[c]["outPb"])
            .transpose(1, 3, 2, 0, 4)
            .reshape(K, N)
            .astype(np.float32)
        )
        acc += Pc
    out = acc.T + np.asarray(b, np.float32)[None, :]
    return np.ascontiguousarray(out)
